# revision 53
# baseline (speedup 1.0000x reference)
"""Trainium2 Bass kernel for nn_CrossAttention (dense_transformer).

Per batch-head (8 of them == 8 cores), with x [C=256, N=4096] views:
  K = W1g @ x1 + b1g   -> knat [64, (y x)]     (raw .view semantics)
  Q = W2g @ x1 + b2g   -> qnat [64, (y x)]
  V = W3g @ x2 + b3g   -> vnat [64, (y x)]
  A[i,j] = sum_x K[i,x] Q[j,x]    (i,j over 4096 = (r,y) pairs; contraction
                                   over the 64 spatial x columns)
  P = exp(A - 64)      (global softmax; the constant shift is exact, and the
                        1/S normalization + b4 bias are applied on the host
                        during unshard: out_dev = W4g @ O_raw, plus the
                        per-core row-sum vector)
  O = P @ V;  host: out = sum_g out_dev[g] / S[g] + b4

Key implementation points:
 - The attention index is enumerated in PERMUTED order i' = y*64 + r
   (instead of the reference's r*64 + y).  The global softmax and the P@V
   contraction are permutation invariant, and this order makes every layout
   step a contiguous per-y operation.
 - K^T/Q^T are produced DIRECTLY in transposed [x, (y r)] orientation:
   for each y, out[x, r] = sum_c x1[c, y*64+x] * W[r, c] is a matmul with
   lhsT = an x1 slice and rhs = W12^T (fp16, 1 cycle/row), with the bias
   added by a third rank-1 matmul into the same PSUM accumulator.  No
   natural-layout staging, no separate transpose pass, no DRAM round trips.
 - x1/x2 are loaded with casting f32->fp16 Pool-SWDGE DMAs; only the
   projection runs in fp16 (A itself is f32r: ~0.6% per-entry exp error).
 - exp runs on PSUM groups of 3/3/2 j-tiles (two 3-bank PSUM pools
   ping-ponging + 1 bank for P@V accumulators + 1 prologue/conv bank
   = exactly 8 banks), bf16 output feeding the P@V matmuls.
 - P@V is computed as [128, 65]-output matmuls (lhsT = exp-tile chunk,
   rhs = V' tile), so O tiles land in natural [i', x] orientation; row sums
   come for free from a ones-column appended to V'.
 - The final 1x1 conv runs PER I-BLOCK during attention (unscaled), so the
   epilogue after the last exp is only a few microseconds.
 - The Activation engine runs NOTHING but exp (its queue issues no DMAs).
"""

import numpy as np

import concourse.bass as bass
import concourse.mybir as mybir
import concourse.tile as tile
from concourse import bacc
from concourse.bass_utils import run_bass_kernel_spmd

F32 = mybir.dt.float32
F32R = mybir.dt.float32r
F16 = mybir.dt.float16
BF16 = mybir.dt.bfloat16
AF = mybir.ActivationFunctionType


def _r(ap):
    return ap.bitcast(F32R)


C = 256          # channels
N = 4096         # y*x spatial positions
D = 64           # head dim (== x) and channels per head block
NJT = N // 128   # 32 j-tiles of 128
NIB = N // 512   # 8 i-blocks of 512
SHIFT = -64.0    # softmax stabilization shift (constant, shift-invariant)
GROUPS = [3] * 10 + [2]   # j-tiles per exp group (sum 32)

_CACHE = {}


def _build():
    import os
    build_stage = int(os.environ.get("KSTAGE", "3"))
    nc = bacc.Bacc("TRN2", target_bir_lowering=False)
    _emit(nc, build_stage)
    nc.finalize()
    return nc


def _emit(nc, build_stage):
    with tile.TileContext(nc) as tc, \
         tc.tile_pool(name="dram", bufs=1, space="DRAM") as dram, \
         tc.tile_pool(name="persist", bufs=1) as persist:
        # ---- I/O ----
        x1 = dram.tile([C, N], F32, kind="ExternalInput", name="x1",
                       uniquify=False)
        x2 = dram.tile([C, N], F32, kind="ExternalInput", name="x2",
                       uniquify=False)
        w12t = dram.tile([C, 2 * D], F32, kind="ExternalInput", name="w12t",
                         uniquify=False)
        b12c = dram.tile([1, 2 * D], F32, kind="ExternalInput", name="b12c",
                         uniquify=False)
        w3t = dram.tile([C, D], F16, kind="ExternalInput", name="w3t",
                        uniquify=False)
        b3g = dram.tile([D, 1], F32, kind="ExternalInput", name="b3g",
                        uniquify=False)
        w4gt = dram.tile([D, C], F32, kind="ExternalInput", name="w4gt",
                         uniquify=False)
        out = dram.tile([C, N], F32, kind="ExternalOutput", name="out",
                        uniquify=False)
        ssum_d = dram.tile([128, 1], F32, kind="ExternalOutput", name="ssum",
                           uniquify=False)

        # ---- persistent SBUF ----
        kqts = persist.tile([D, 2 * N], F32R)     # [x, (K^T | Q^T)] merged
        vnat = persist.tile([D, N], F32)
        v2 = persist.tile([128, NJT * (D + 1)], BF16)  # V' tiles [j-loc, 65]
        oconv = persist.tile([D, N], F32R)        # O' [cl, (y x)]
        knatq = persist.tile([128, N], F32)       # K rows 0:64, Q rows 64:128
        w12a = persist.tile([C // 2, 2 * D], F32R)
        w12b = persist.tile([C // 2, 2 * D], F32R)
        w3a = persist.tile([C // 2, D], F16)
        w3b = persist.tile([C // 2, D], F16)
        w4sb = persist.tile([D, C], F32R)         # [cl, o] = W4[:, gs].T
        b12s = persist.tile([1, 2 * D], F32R)
        b3s = persist.tile([D, 1], F32)
        ones5 = persist.tile([1, 512], F32)
        ones5r = persist.tile([1, 512], F32R)
        ident = persist.tile([128, 128], F32)
        shift = persist.tile([128, 1], F32)
        acc = persist.tile([128, 1], F32)         # running exp row sums
        dummy = persist.tile([1, 1], F32)

        v2v = v2.rearrange("p (j c) -> p j c", c=D + 1)
        ktv = kqts.rearrange("p (k n) -> p k n", k=2)   # k=0: K^T, k=1: Q^T
        x1v = x1.rearrange("c (y x) -> c y x", x=D)
        x2v = x2.rearrange("c (y x) -> c y x", x=D)

        # ---- constants ----
        nc.gpsimd.memset(shift[:], SHIFT)
        nc.gpsimd.memset(acc[:], 0.0)
        from concourse.masks import make_identity
        make_identity(nc, ident[:])
        nc.gpsimd.memset(ones5[:], 1.0)
        nc.vector.tensor_copy(ones5r[:], ones5[:])
        ones_j = persist.tile([128, NJT], F32)
        nc.gpsimd.memset(ones_j[:], 1.0)
        nc.vector.tensor_copy(v2v[:, :, D], ones_j[:])

        # dummy exp very early: pulls the ACT table load off the critical path
        nc.scalar.activation(dummy[:], shift[0:1, 0:1], AF.Exp,
                             bias=0.0, scale=1.0)
        nc.scalar.dma_start(b12s[:], _r(b12c[:]))
        nc.sync.dma_start(w12a[:], _r(w12t[0:128, :]))
        nc.gpsimd.dma_start(w12b[:], _r(w12t[128:256, :]))

        # x loads: f32 chunks on the SP queue (first 1024 cols, then rest)
        xpool_cm = tc.tile_pool(name="xsb", bufs=1)
        xpool = xpool_cm.__enter__()
        x1a = xpool.tile([128, N], F32R)
        x1b = xpool.tile([128, N], F32R)
        x2a = xpool.tile([128, N], F16)
        x2b = xpool.tile([128, N], F16)

        # PSUM pools are a stack (LIFO release): open ovp (whole kernel),
        # then kqT/kqS (phase A), then prol (prologue, first to close).
        ovp_cm = tc.tile_pool(name="ovp", bufs=1, space="PSUM")
        ovp = ovp_cm.__enter__()
        kqT_cm = tc.tile_pool(name="kqT", bufs=1, space="PSUM")
        kqT = kqT_cm.__enter__()
        kqS_cm = tc.tile_pool(name="kqS", bufs=1, space="PSUM")
        kqS = kqS_cm.__enter__()
        prol_cm = tc.tile_pool(name="prol", bufs=1, space="PSUM")
        prol = prol_cm.__enter__()

        def emit_x_dmas(t, ch):
            # a dma_start holds its issuing engine for the whole transfer,
            # so spread: x1 head on the (pre-exp idle) Act queue, x1 tail on
            # SP/Pool in f32; x2 as fp16 cast-DMAs on Pool (half the bytes),
            # in four pieces interleaved with the transpose-drain copies.
            if t == 1 and ch == 0:
                nc.scalar.dma_start(x1a[:, 0:1024], _r(x1[0:128, 0:1024]))
                nc.gpsimd.dma_start(x1b[:, 0:1024], _r(x1[128:256, 0:1024]))
            elif t == 1 and ch == 1:
                nc.sync.dma_start(x1a[:, 1024:2048], _r(x1[0:128, 1024:2048]))
                nc.scalar.dma_start(x1b[:, 1024:2048],
                                    _r(x1[128:256, 1024:2048]))
            elif t == 1:
                s = slice(1024 * ch, 1024 * (ch + 1))
                nc.sync.dma_start(x1a[:, s], _r(x1[0:128, s]))
                nc.gpsimd.dma_start(x1b[:, s], _r(x1[128:256, s]))
            else:
                half, piece = ch % 2, ch // 2
                s = slice(piece * 2048, (piece + 1) * 2048)
                dst = x2a if half == 0 else x2b
                sr = x2[0:128, s] if half == 0 else x2[128:256, s]
                nc.gpsimd.dma_start(dst[:, s], sr)
                if ch == 0:
                    nc.sync.dma_start(b3s[:], b3g[:])
                    nc.sync.dma_start(w3a[:], w3t[0:128, :])
                    nc.sync.dma_start(w3b[:], w3t[128:256, :])
                    nc.sync.dma_start(w4sb[:], _r(w4gt[:]))

        for _ in range(40):
            wtp = prol.tile([D, 128], F32, name="pp", bufs=3)
            nc.tensor.transpose(wtp[:, 0:D], ident[0:D, 0:D],
                                ident[0:D, 0:D])

        def emit_pj(i):
            # K/Q projection for cols [512i, 512i+512); bias joins the
            # contraction as a rank-1 matmul (no elementwise pass needed)
            s = slice(i * 512, (i + 1) * 512)
            pp = prol.tile([128, 512], F32, name="pp", bufs=3)
            nc.tensor.matmul(pp[:], w12a[:], x1a[:, s],
                             start=True, stop=False)
            nc.tensor.matmul(pp[:], w12b[:], x1b[:, s],
                             start=False, stop=False)
            nc.tensor.matmul(pp[:], b12s[:], ones5r[:],
                             start=False, stop=True)
            nc.vector.tensor_copy(knatq[:, s], pp[:])

        def emit_tr(y0, y1):
            # PE transposes of knatq in y-pairs; one DVE drain per pair
            for y in range(y0, y1, 2):
                tp2 = prol.tile([D, 256], F32, name="pp", bufs=3)
                nc.tensor.transpose(tp2[:, 0:128],
                                    knatq[:, y * D:(y + 1) * D], ident[:])
                nc.tensor.transpose(tp2[:, 128:256],
                                    knatq[:, (y + 1) * D:(y + 2) * D],
                                    ident[:])
                dstv = ktv[:, :, y * D:(y + 2) * D].rearrange(
                    "p k (y2 r) -> p k y2 r", y2=2)
                nc.vector.tensor_copy(
                    dstv, tp2.rearrange("p (y2 k r) -> p k y2 r", y2=2, k=2))

        def emit_vj(i):
            # V projection for cols [512i, 512i+512); rank-1 bias included
            s = slice(i * 512, (i + 1) * 512)
            pv = prol.tile([128, 512], F32, name="pp", bufs=3)
            nc.tensor.matmul(pv[0:D, :], w3a[:], x2a[:, s],
                             start=True, stop=False)
            nc.tensor.matmul(pv[0:D, :], w3b[:], x2b[:, s],
                             start=False, stop=True)
            nc.vector.tensor_scalar_add(vnat[:, s], pv[0:D, :], b3s[:])

        def emit_vc(y0, y1):
            # V' staging: all-SBUF Pool copies with bf16 cast
            for y in range(y0, y1):
                ysl = slice(y * D, (y + 1) * D)
                p0 = (y % 2) * D
                nc.gpsimd.tensor_copy(v2v[p0:p0 + D, y // 2, 0:D],
                                      vnat[:, ysl])

        # ---- attention ----
        # Phase A (during the prologue): i-block 0 in 16 groups of 2 j-tiles
        # on two 2-bank PSUM pools; prologue pool holds 3 banks.
        # Phase B (steady state): i-blocks 1-7 in 3/3/2 groups on two 3-bank
        # pools + 1 conv bank.  Both phases: +1 bank of P@V accumulators.
        with tc.tile_pool(name="stgp", bufs=2) as stgp, \
             tc.tile_pool(name="otp", bufs=2) as otp, \
             tc.tile_pool(name="ptA", bufs=22) as ptA, \
             tc.tile_pool(name="ptp", bufs=4) as ptp:
            sched = [(0, 2 * k, 2) for k in range(16)]
            sched += [(1, 2 * k, 2) for k in range(4)]
            sched += [(1, 8 + 3 * i, 3) for i in range(8)]
            for ib in range(2, NIB):
                jt = 0
                for sz in GROUPS:
                    sched.append((ib, jt, sz))
                    jt += sz
            ov4s = {}
            cvp = None
            cvp_cm = None
            kqA = kqB = None

            deferred_conv = []

            def emit_conv(ib):
                isl = slice(ib * 512, (ib + 1) * 512)
                for oc in range(2):
                    cp = cvp.tile([128, 512], F32, name="cvps")
                    nc.tensor.matmul(
                        cp[:], w4sb[:, oc * 128:(oc + 1) * 128],
                        oconv[:, isl], start=True, stop=True)
                    ot = otp.tile([128, 512], F32, name="cvsb")
                    nc.vector.tensor_copy(ot[:], cp[:])
                    if (ib + oc) % 2 == 0:
                        nc.sync.dma_start(
                            out[oc * 128:(oc + 1) * 128, isl], ot[:])
                    else:
                        nc.gpsimd.dma_start(
                            out[oc * 128:(oc + 1) * 128, isl], ot[:])

            def emit_conv_fast(ib):
                # last i-block: halve the conv so drains/stores pipeline
                isl = slice(ib * 512, (ib + 1) * 512)
                for oc in range(2):
                    for hh in range(2):
                        s = slice(ib * 512 + hh * 256,
                                  ib * 512 + (hh + 1) * 256)
                        cp = cvp.tile([128, 256], F32, name="cvps")
                        nc.tensor.matmul(
                            cp[:], w4sb[:, oc * 128:(oc + 1) * 128],
                            oconv[:, s], start=True, stop=True)
                        ot = otp.tile([128, 256], F32, name="cvsbf")
                        nc.vector.tensor_copy(ot[:], cp[:])
                        if (oc + hh) % 2 == 0:
                            nc.sync.dma_start(
                                out[oc * 128:(oc + 1) * 128, s], ot[:])
                        else:
                            nc.gpsimd.dma_start(
                                out[oc * 128:(oc + 1) * 128, s], ot[:])

            def emit_pv(ib, jt0, sz, ptv):
                if ib not in ov4s:
                    ov4s[ib] = ovp.tile([128, 4 * (D + 1)], F32, name="ov4")
                ovv = ov4s[ib].rearrange("p (i c) -> p i c", c=D + 1)
                for gi in range(sz):
                    j = jt0 + gi
                    for ic in range(4):
                        nc.tensor.matmul(
                            ovv[:, ic, :],
                            ptv[:, gi, ic * 128:(ic + 1) * 128],
                            v2v[:, j, :],
                            start=(j == 0), stop=(j == NJT - 1),
                            skip_group_check=True)
                if jt0 + sz == NJT:
                    # i-block done: stage PSUM out, scatter into oconv, then
                    # run this i-block's slice of the final conv (unscaled).
                    stg = stgp.tile([128, 4 * (D + 1)], F32, name="ovstg")
                    nc.vector.tensor_copy(stg[:], ov4s[ib][:])
                    sgv = stg.rearrange("p (i c) -> p i c", c=D + 1)
                    if ib == NIB - 1:
                        for ic in range(4):
                            nc.vector.tensor_add(acc[:], acc[:],
                                                 sgv[:, ic, D:D + 1])
                        # row sums final: export in parallel with the conv
                        nc.gpsimd.dma_start(ssum_d[:], acc[:])
                    for ic in range(4):
                        for h1 in range(2):
                            y = ib * 8 + ic * 2 + h1
                            eng = nc.vector if h1 == 0 else nc.gpsimd
                            eng.tensor_copy(
                                oconv[:, y * D:(y + 1) * D],
                                sgv[h1 * D:(h1 + 1) * D, ic, 0:D])
                        if ib < NIB - 1:
                            nc.vector.tensor_add(acc[:], acc[:],
                                                 sgv[:, ic, D:D + 1])
                    del ov4s[ib]
                    if cvp is None:
                        deferred_conv.append(ib)
                    elif ib == NIB - 1:
                        emit_conv_fast(ib)
                    else:
                        emit_conv(ib)

            # prologue interleave plan, keyed by phase-A group index k
            # (group k covers j-tiles 2k, 2k+1):
            # kq(k) reads qtsb y <= 4k+3 and ktsb y < 8; pv(k) at k+1 reads
            # v2 y <= 4k+3.
            pre = {
                0: [("x1", 0), ("x1", 1), ("pj", 0), ("tr", 0, 8)],
                1: [("pj", 1), ("tr", 8, 16)],
                3: [("pj", 2), ("tr", 16, 24), ("x1", 2)],
                4: [("pj", 3), ("tr", 24, 32), ("x1", 3)],
                5: [("pj", 4), ("tr", 32, 40), ("x2", 0)],
                6: [("pj", 5), ("tr", 40, 48), ("x2", 1)],
                7: [("pj", 6), ("tr", 48, 56), ("x2", 2)],
                8: [("pj", 7), ("tr", 56, 64), ("x2", 3)],
                11: [("vj", 0), ("vc", 0, 8)],
                12: [("vj", 1), ("vc", 8, 16)],
                13: [("vj", 2), ("vc", 16, 24)],
                14: [("vj", 3), ("vc", 24, 32)],
                16: [("vj", 4), ("vc", 32, 40)],
                17: [("vj", 5), ("vc", 40, 48)],
                18: [("vj", 6), ("vc", 48, 56)],
                19: [("vj", 7), ("vc", 56, 64)],
            }
            NPA = 20   # phase-A group count (2 j-tiles each)
            CAP_A = 99  # defer all P@V to phase B
            pend = []
            for gidx, (ib, jt0, sz) in enumerate(sched):
                for step in pre.get(gidx, ()):
                    if step[0] == "x1":
                        emit_x_dmas(1, step[1])
                    elif step[0] == "x2":
                        emit_x_dmas(2, step[1])
                    elif step[0] == "pj":
                        emit_pj(step[1])
                    elif step[0] == "tr":
                        emit_tr(step[1], step[2])
                    elif step[0] == "vj":
                        emit_vj(step[1])
                    else:
                        emit_vc(step[1], step[2])
                if gidx == NPA:
                    # phase B: 3-bank kq pools + the conv bank
                    prol_cm.__exit__(None, None, None)
                    kqS_cm.__exit__(None, None, None)
                    kqT_cm.__exit__(None, None, None)
                    kqA_cm = tc.tile_pool(name="kqA", bufs=1, space="PSUM")
                    kqA = kqA_cm.__enter__()
                    kqB_cm = tc.tile_pool(name="kqB", bufs=1, space="PSUM")
                    kqB = kqB_cm.__enter__()
                    cvp_cm = tc.tile_pool(name="cvp", bufs=1, space="PSUM")
                    cvp = cvp_cm.__enter__()
                if gidx == NPA + 2 and deferred_conv:
                    for dib in deferred_conv:
                        emit_conv(dib)
                    deferred_conv.clear()
                isl = slice(ib * 512, (ib + 1) * 512)
                if gidx < NPA:
                    pool = kqS if gidx % 2 == 0 else kqT
                    kq = pool.tile([128, 2 * 512], F32, name="kq")
                    pt = ptA.tile([128, 2 * 512], BF16, name="ptA")
                else:
                    pool = kqA if gidx % 2 == 0 else kqB
                    kq = pool.tile([128, 3 * 512], F32, name="kq")
                    pt = ptp.tile([128, 3 * 512], BF16, name="pt")
                for gi in range(sz):
                    jcol = (jt0 + gi) * 128
                    nc.tensor.matmul(
                        kq[:, gi * 512:(gi + 1) * 512],
                        ktv[:, 1, jcol:jcol + 128],
                        ktv[:, 0, isl], start=True, stop=True)
                nc.scalar.activation(pt[:, 0:sz * 512], kq[:, 0:sz * 512],
                                     AF.Exp, bias=shift[:], scale=1.0)
                pend.append((ib, jt0, sz,
                             pt.rearrange("p (g s) -> p g s", s=512)))
                if gidx < NPA:
                    while len(pend) > CAP_A:
                        emit_pv(*pend.pop(0))
                else:
                    flushed = 0
                    while len(pend) > 1 and flushed < 2:
                        emit_pv(*pend.pop(0))
                        flushed += 1
            while pend:
                emit_pv(*pend.pop(0))

            cvp_cm.__exit__(None, None, None)
            kqB_cm.__exit__(None, None, None)
            kqA_cm.__exit__(None, None, None)
            ovp_cm.__exit__(None, None, None)
        xpool_cm.__exit__(None, None, None)


def get_nc():
    if "nc" not in _CACHE:
        _CACHE["nc"] = _build()
    return _CACHE["nc"]


def make_in_maps(input_tensor1, input_tensor2, W1, b1, W2, b2, W3, b3, W4, b4):
    x1 = np.ascontiguousarray(np.asarray(input_tensor1, dtype=np.float32))
    x2 = np.ascontiguousarray(np.asarray(input_tensor2, dtype=np.float32))
    W1, W2, W3, W4 = (np.asarray(w, dtype=np.float32) for w in (W1, W2, W3, W4))
    b1, b2, b3 = (np.asarray(b, dtype=np.float32) for b in (b1, b2, b3))
    in_maps = []
    for p in range(8):
        b, g = p // 4, p % 4
        gs = slice(g * D, (g + 1) * D)
        w12 = np.concatenate([W1[gs, :].T, W2[gs, :].T], axis=1)
        in_maps.append({
            "x1": x1[b].reshape(C, N),
            "x2": x2[b].reshape(C, N),
            "w12t": np.ascontiguousarray(w12),
            "b12c": np.concatenate([b1[gs], b2[gs]]).reshape(1, 2 * D).copy(),
            "w3t": np.ascontiguousarray(W3[gs, :].T).astype(np.float16),
            "b3g": b3[gs].reshape(D, 1).copy(),
            "w4gt": np.ascontiguousarray(W4[:, gs].T),
        })
    return in_maps


def kernel(input_tensor1, input_tensor2, W1, b1, W2, b2, W3, b3, W4, b4):
    nc = get_nc()
    in_maps = make_in_maps(input_tensor1, input_tensor2,
                           W1, b1, W2, b2, W3, b3, W4, b4)
    res = run_bass_kernel_spmd(nc, in_maps, core_ids=list(range(8)))
    b4 = np.asarray(b4, dtype=np.float32)
    full = np.empty((2, C, 64, 64), dtype=np.float32)
    for b in range(2):
        acc = np.zeros((C, N), dtype=np.float64)
        for g in range(4):
            p = b * 4 + g
            s = float(np.asarray(res.results[p]["ssum"], dtype=np.float64).sum())
            acc += np.asarray(res.results[p]["out"], dtype=np.float64) / s
        acc += b4[:, None]
        # device layout is [o, y*64+x] == [o, h, w]
        full[b] = acc.reshape(C, 64, 64).astype(np.float32)
    return full
